# revision 1
# baseline (speedup 1.0000x reference)
"""Trainium2 Bass kernel for CRF negative log-likelihood (nn_CRF).

Strategy (see inline comments):
  - data-parallel over batch: 8 cores x 16 sequences each
  - forward algorithm in the exp domain: X_t = d_t * (E^T X_{t-1}) with
    E = exp(transitions); constant per-step rescale exp(-C0) folded into the
    emission tensor; periodic per-batch renormalization (colsum via ones
    matmul) every RENORM steps keeps fp32 in range.
  - masking via an absorbing-STOP construction: E[STOP,STOP]:=1, active steps
    emit d[STOP]=0, frozen steps emit d=onehot(STOP).  The final answer for
    every sequence is X_final[STOP] after one virtual terminal step, so the
    variable lengths never require per-step blending.
  - latency: the 256-step serial chain is split into a forward half
    (t=0..128) and an independent backward half (beta from t=256 down to 129);
    the two chains interleave on PE/DVE and the answer is the per-column dot
    product of the halves at the midpoint.
  - gold path score via one-hot is_equal tensors (GPSIMD) contracted with
    feats (DVE) and pair/end counts (PE matmuls against the one-hots).
  - device emits small per-core partials (raw renorm scales, midpoint dot,
    gold partial sums); the host does the final log/sum ("all-reduce").
"""

import numpy as np

TAG = 52
START, STOP = TAG - 2, TAG - 1
B, S = 128, 256
NCORES = 8
BL = B // NCORES            # 16 sequences per core
MID = 128                   # forward/backward split point
RENORM = 64                 # renormalize every this many steps
C0 = 4.9                    # constant per-step rescale (nats)
MGATE = 64.0                # mask gate constant (exp(-64) == 0 in fp32)
CHUNK = 64                  # emission build chunk (along t)
M32 = (S * BL) // 128       # 32 free columns for (128, M32) gold layout

_CACHE: dict = {}


def _build_nc(debug: bool = False):
    import os
    parts = os.environ.get("KPARTS", "all")   # all | scan | gold | setup
    do_scan = parts in ("all", "scan")
    do_gold = parts in ("all", "gold")
    import concourse.bass as bass
    import concourse.mybir as mybir
    import concourse.tile as tile
    from concourse import bacc

    f32 = mybir.dt.float32
    nc = bacc.Bacc("TRN2", target_bir_lowering=False, debug=debug)

    # ---- external inputs (per-core shards, host-marshalled layouts) ----
    featsT = nc.dram_tensor("featsT", (TAG, S, BL), f32, kind="ExternalInput")
    featsN = nc.dram_tensor("featsN", (128, M32, TAG), f32, kind="ExternalInput")
    mtb = nc.dram_tensor("mtb", (S, BL), f32, kind="ExternalInput")
    maskf = nc.dram_tensor("maskf", (128, M32), f32, kind="ExternalInput")
    mnextf = nc.dram_tensor("mnextf", (128, M32), f32, kind="ExternalInput")
    tagsf = nc.dram_tensor("tagsf", (128, M32), f32, kind="ExternalInput")
    prevf = nc.dram_tensor("prevf", (128, M32), f32, kind="ExternalInput")
    transr = nc.dram_tensor("transr", (TAG, TAG), f32, kind="ExternalInput")
    iotaf = nc.dram_tensor("iotaf", (128, TAG), f32, kind="ExternalInput")
    ident = nc.dram_tensor("ident", (TAG, TAG), f32, kind="ExternalInput")
    colconsts = nc.dram_tensor("colconsts", (TAG, 2), f32, kind="ExternalInput")

    # ---- external outputs ----
    # out_scan column blocks of BL: 0=Ssum, 1..4 = renorm scales
    out_scan = nc.dram_tensor("out_scan", (1, 8 * BL), f32, kind="ExternalOutput")
    # out_gold cols: 0 = per-(b,s)-row emit partials, 1 = trans*count partials,
    #                2 = end-transition partials
    out_gold = nc.dram_tensor("out_gold", (128, 4), f32, kind="ExternalOutput")

    AL = mybir.AluOpType

    with tile.TileContext(nc) as tc:
        with (
            tc.tile_pool(name="persist", bufs=1) as persist,
            tc.tile_pool(name="chunks", bufs=2) as chunks,
            tc.tile_pool(name="state", bufs=3) as statep,
            tc.tile_pool(name="small", bufs=2) as small,
            tc.tile_pool(name="gold", bufs=1) as goldp,
            tc.tile_pool(name="psum", bufs=1, space="PSUM") as psum,
            tc.tile_pool(name="psumg", bufs=1, space="PSUM") as psumg,
        ):
            # ================= constants / setup =================
            tr_sb = persist.tile([TAG, TAG], f32, name="tr_sb", tag="tr_sb")
            nc.sync.dma_start(out=tr_sb, in_=transr[:, :])
            id_sb = persist.tile([TAG, TAG], f32, name="id_sb", tag="id_sb")
            nc.sync.dma_start(out=id_sb, in_=ident[:, :])

            ones_col = persist.tile([TAG, 1], f32, name="ones_col", tag="ones_col")
            nc.vector.memset(ones_col, 1.0)
            ones_row = persist.tile([1, TAG], f32, name="ones_row", tag="ones_row")
            nc.vector.memset(ones_row, 1.0)
            colc = persist.tile([TAG, 2], f32, name="colc", tag="colc")
            nc.sync.dma_start(out=colc, in_=colconsts[:, :])
            sgate = colc[:, 0:1]
            biasc = colc[:, 1:2]

            # patch trans[STOP, STOP] = 0 (so exp gives 1) via tiny const DMA
            # (ident[0, 1] == 0.0); engines cannot address start partition 51,
            # DMA can.
            nc.sync.dma_start(
                out=tr_sb[STOP : STOP + 1, STOP : STOP + 1], in_=ident[0:1, 1:2]
            )
            # Etil = exp(trans); Etil[STOP, STOP] = exp(0) = 1
            Etil = persist.tile([TAG, TAG], f32, name="Etil", tag="Etil")
            nc.scalar.activation(
                out=Etil, in_=tr_sb, func=mybir.ActivationFunctionType.Exp
            )
            # EtilT = Etil^T (PE transpose through PSUM)
            ps_t = psum.tile([TAG, TAG], f32, name="ps_t", tag="ps_b")
            nc.tensor.transpose(ps_t, Etil, id_sb)
            EtilT = persist.tile([TAG, TAG], f32, name="EtilT", tag="EtilT")
            nc.vector.tensor_copy(EtilT, ps_t)

            # ================= emission tensor D (TAG, S, BL) =================
            if not do_scan:
                nc.vector.memset(stage_scan_dummy___ := None or persist.tile(
                    [1, 8 * BL], f32, name="stage_scan", tag="stage_scan"), 1.0)
                nc.sync.dma_start(out=out_scan[:, :], in_=stage_scan_dummy___)
            if do_scan:
                D = persist.tile([TAG, S, BL], f32, name="D", tag="D")
                # build in t-chunks; order 0,3,1,2 so fwd (chunk0) and bwd (chunk3)
                # can start as early as possible
                for c in (0, 3, 1, 2):
                    t0 = c * CHUNK
                    ft = chunks.tile([TAG, CHUNK, BL], f32, name="ft", tag="ft")
                    nc.sync.dma_start(out=ft, in_=featsT[:, t0 : t0 + CHUNK, :])
                    mrep = chunks.tile([TAG, CHUNK, BL], f32, name="mrep", tag="mrep")
                    src = bass.AP(
                        tensor=mtb,
                        offset=t0 * BL,
                        ap=[[0, TAG], [BL, CHUNK], [1, BL]],
                    )
                    nc.sync.dma_start(out=mrep, in_=src)
                    # ft <- (mrep * sgate) + ft
                    nc.vector.scalar_tensor_tensor(
                        out=ft, in0=mrep, scalar=sgate, in1=ft, op0=AL.mult, op1=AL.add
                    )
                    nc.scalar.activation(
                        out=D[:, t0 : t0 + CHUNK, :],
                        in_=ft,
                        func=mybir.ActivationFunctionType.Exp,
                        bias=biasc,
                    )

                # ================= scan state init =================
                X = statep.tile([TAG, BL], f32, name="X", tag="X")
                nc.vector.tensor_scalar_mul(
                    out=X, in0=D[:, 0, :], scalar1=EtilT[:, START : START + 1]
                )
                BT = statep.tile([TAG, BL], f32, name="BT", tag="BT")
                nc.vector.memset(BT, 1.0)
                nc.vector.tensor_scalar_mul(
                    out=BT, in0=BT, scalar1=Etil[:, STOP : STOP + 1]
                )

                stage_scan = persist.tile(
                    [1, 8 * BL], f32, name="stage_scan", tag="stage_scan"
                )
                nc.vector.memset(stage_scan, 0.0)

                def renorm(V, srow, blockname):
                    """V (TAG, BL) sbuf -> V / colsum(V); stage raw colsum in
                    stage_scan[srow]."""
                    ps_s = psum.tile([1, BL], f32, name=f"ps_s_{blockname}", tag="ps_s")
                    nc.tensor.matmul(ps_s, ones_col, V, start=True, stop=True)
                    nc.vector.tensor_copy(stage_scan[0:1, srow * BL : (srow + 1) * BL], ps_s)
                    rcp = small.tile([1, BL], f32, name=f"rcp_{blockname}", tag="rcp")
                    nc.vector.reciprocal(rcp, ps_s)
                    ps_b = psum.tile([TAG, BL], f32, name=f"ps_b_{blockname}", tag="ps_b")
                    nc.tensor.matmul(ps_b, ones_row, rcp, start=True, stop=True)
                    V2 = statep.tile([TAG, BL], f32, name=f"rn_{blockname}", tag=blockname)
                    nc.vector.tensor_mul(V2, V, ps_b)
                    return V2

                # ================= interleaved fwd/bwd scans =================
                # fwd: X_t = d_t * (Etil^T X_{t-1}),          t = 1..MID
                # bwd: beta_{t-1} = Etil (d_t * beta_t),      t = 255..MID+1
                nren_f = 0
                nren_b = 0
                for k in range(1, MID + 1):
                    # ---- forward step t = k ----
                    ps_f = psum.tile([TAG, BL], f32, name="ps_f", tag="ps_f", bufs=2)
                    nc.tensor.matmul(ps_f, Etil, X, start=True, stop=True)
                    Xn = statep.tile([TAG, BL], f32, name="Xn", tag="X")
                    nc.vector.tensor_mul(Xn, ps_f, D[:, k, :])
                    X = Xn
                    if k % RENORM == 0 or k == MID:
                        if k % RENORM == 0 and k != MID:
                            X = renorm(X, 1, "X")
                            nren_f += 1
                        else:
                            X = renorm(X, 2, "X")
                            nren_f += 1
                    # ---- backward step t = S - k (skip when t <= MID) ----
                    t = S - k
                    if t > MID:
                        bt = statep.tile([TAG, BL], f32, name="bt", tag="BT")
                        nc.vector.tensor_mul(bt, BT, D[:, t, :])
                        ps_bw = psum.tile([TAG, BL], f32, name="ps_bw", tag="ps_bw", bufs=2)
                        nc.tensor.matmul(ps_bw, EtilT, bt, start=True, stop=True)
                        BTn = statep.tile([TAG, BL], f32, name="BTn", tag="BT")
                        nc.vector.tensor_copy(BTn, ps_bw)
                        BT = BTn
                        if k % RENORM == 0:
                            BT = renorm(BT, 3, "BT")
                            nren_b += 1
                        elif t == MID + 1:
                            BT = renorm(BT, 4, "BT")
                            nren_b += 1

                # ================= midpoint combine =================
                P = statep.tile([TAG, BL], f32, name="P", tag="X")
                nc.vector.tensor_mul(P, X, BT)
                ps_c = psum.tile([1, BL], f32, name="ps_c", tag="ps_s")
                nc.tensor.matmul(ps_c, ones_col, P, start=True, stop=True)
                nc.vector.tensor_copy(stage_scan[0:1, 0:BL], ps_c)
                nc.sync.dma_start(out=out_scan[:, :], in_=stage_scan)

            # ================= gold score =================
            if not do_gold:
                gdum = goldp.tile([128, 4], f32, name="stage_gold", tag="stage_gold")
                nc.vector.memset(gdum, 1.0)
                nc.sync.dma_start(out=out_gold[:, :], in_=gdum)
            if do_gold:
                FN = goldp.tile([128, M32, TAG], f32, name="FN", tag="FN")
                nc.sync.dma_start(out=FN, in_=featsN[:, :, :])
                MK = goldp.tile([128, M32], f32, name="MK", tag="MK")
                nc.sync.dma_start(out=MK, in_=maskf[:, :])
                MN = goldp.tile([128, M32], f32, name="MN", tag="MN")
                nc.sync.dma_start(out=MN, in_=mnextf[:, :])
                TGf = goldp.tile([128, M32], f32, name="TGf", tag="TGf")
                nc.sync.dma_start(out=TGf, in_=tagsf[:, :])
                PV = goldp.tile([128, M32], f32, name="PV", tag="PV")
                nc.sync.dma_start(out=PV, in_=prevf[:, :])
                IO = goldp.tile([128, TAG], f32, name="IO", tag="IO")
                nc.sync.dma_start(out=IO, in_=iotaf[:, :])

                # tags_m = (tags + 1) * mask - 1
                TGM = goldp.tile([128, M32], f32, name="TGM", tag="TGM")
                nc.vector.tensor_scalar_add(out=TGM, in0=TGf, scalar1=1.0)
                nc.vector.tensor_mul(TGM, TGM, MK)
                nc.vector.tensor_scalar_add(out=TGM, in0=TGM, scalar1=-1.0)
                # w_last = mask - mask_next
                WL = goldp.tile([128, M32], f32, name="WL", tag="WL")
                nc.vector.tensor_sub(WL, MK, MN)

                def bcast_cmp(out_t, vals):
                    # out[p, m, j] = (vals[p, m] == iota[p, j])
                    v3 = bass.AP(
                        tensor=vals.tensor,
                        offset=vals.offset,
                        ap=[vals.ap[0], vals.ap[1], [0, TAG]],
                    )
                    i3 = bass.AP(
                        tensor=IO.tensor,
                        offset=IO.offset,
                        ap=[IO.ap[0], [0, M32], IO.ap[1]],
                    )
                    nc.vector.tensor_tensor(out=out_t, in0=v3, in1=i3, op=AL.is_equal)

                Y = goldp.tile([128, M32, TAG], f32, name="Y", tag="Y")
                bcast_cmp(Y, TGM)
                YP = goldp.tile([128, M32, TAG], f32, name="YP", tag="YP")
                bcast_cmp(YP, PV)

                stage_gold = goldp.tile([128, 4], f32, name="stage_gold", tag="stage_gold")
                nc.vector.memset(stage_gold, 0.0)

                # emit partials: sum_j (Y * featsN) per (b,s)-row
                scrap = goldp.tile([128, M32, TAG], f32, name="scrap", tag="scrap")
                nc.vector.tensor_mul(scrap, Y, FN)
                nc.vector.tensor_reduce(
                    out=stage_gold[:, 0:1],
                    in_=scrap,
                    axis=mybir.AxisListType.XY,
                    op=AL.add,
                )

                # pair counts: cnt[i, j] = sum_bs YP[bs, i] * Y[bs, j]
                ps_cnt = psumg.tile([TAG, TAG], f32, name="ps_cnt", tag="ps_cnt")
                for m in range(M32):
                    nc.tensor.matmul(
                        ps_cnt,
                        YP[:, m, :],
                        Y[:, m, :],
                        start=(m == 0),
                        stop=(m == M32 - 1),
                    )
                # trans partials: sum_j cnt[i, j] * trans[i, j] per i
                scrap2 = goldp.tile([TAG, TAG], f32, name="scrap2", tag="scrap2")
                nc.vector.tensor_mul(scrap2, ps_cnt, tr_sb)
                nc.vector.tensor_reduce(
                    out=stage_gold[0:TAG, 1:2],
                    in_=scrap2,
                    axis=mybir.AxisListType.X,
                    op=AL.add,
                )

                # end counts: endcnt[j] = sum_bs Y[bs, j] * w_last[bs]
                ps_end = psumg.tile([TAG, 1], f32, name="ps_end", tag="ps_end")
                for m in range(M32):
                    nc.tensor.matmul(
                        ps_end,
                        Y[:, m, :],
                        WL[:, m : m + 1],
                        start=(m == 0),
                        stop=(m == M32 - 1),
                    )
                nc.vector.tensor_mul(
                    stage_gold[0:TAG, 2:3], ps_end, tr_sb[:, STOP : STOP + 1]
                )

                nc.sync.dma_start(out=out_gold[:, :], in_=stage_gold)

    nc.compile()
    return nc


def _prep_core_inputs(feats, transitions, mask, tags, core):
    """Layout-only host marshalling of the core's batch shard."""
    f32 = np.float32
    sl = slice(core * BL, (core + 1) * BL)
    f = np.ascontiguousarray(feats[sl]).astype(f32, copy=False)   # (BL,S,T)
    m = mask[sl].astype(f32)                                      # (BL,S)
    tg = tags[sl].astype(f32)                                     # (BL,S)

    featsT = np.ascontiguousarray(f.transpose(2, 1, 0)).copy()    # (T,S,BL)
    featsT[STOP] = 0.0
    featsN = np.ascontiguousarray(f.reshape(BL * S, TAG)).reshape(128, M32, TAG)
    mtb = np.ascontiguousarray(m.T)                               # (S,BL)
    maskf = m.reshape(128, M32)
    mnext = np.concatenate([m[:, 1:], np.zeros((BL, 1), f32)], axis=1)
    mnextf = mnext.reshape(128, M32)
    tagsf = tg.reshape(128, M32)
    prev = np.concatenate([np.full((BL, 1), START, f32), tg[:, :-1]], axis=1)
    prevf = prev.reshape(128, M32)
    transr = transitions.astype(f32, copy=False)
    iotaf = np.broadcast_to(np.arange(TAG, dtype=f32), (128, TAG)).copy()
    ident = np.eye(TAG, dtype=f32)
    colconsts = np.zeros((TAG, 2), f32)
    colconsts[:, 0] = MGATE
    colconsts[STOP, 0] = -MGATE
    colconsts[:, 1] = -(MGATE + C0)
    colconsts[STOP, 1] = 0.0
    return {
        "featsT": np.ascontiguousarray(featsT),
        "featsN": np.ascontiguousarray(featsN),
        "mtb": mtb,
        "maskf": np.ascontiguousarray(maskf),
        "mnextf": np.ascontiguousarray(mnextf),
        "tagsf": np.ascontiguousarray(tagsf),
        "prevf": np.ascontiguousarray(prevf),
        "transr": np.ascontiguousarray(transr),
        "iotaf": iotaf,
        "ident": ident,
        "colconsts": colconsts,
    }


def _combine(results, mask):
    """Host-side unshard: logs of staged scales + partial sums -> scalar."""
    f32 = np.float32
    lengths = mask.astype(np.int64).sum(axis=1)  # (B,)
    fwd = np.float64(0.0)
    gold = np.float64(0.0)
    for core, res in enumerate(results):
        sc = res["out_scan"].astype(np.float64).reshape(8, BL)
        gl = res["out_gold"].astype(np.float64)      # (128, 4)
        ln = np.log(sc[0]) + np.log(sc[1]) + np.log(sc[2]) + np.log(sc[3]) \
            + np.log(sc[4])
        lens = lengths[core * BL : (core + 1) * BL].astype(np.float64)
        fwd += (ln + C0 * lens).sum()
        gold += gl[:, 0].sum() + gl[0:TAG, 1].sum() + gl[0:TAG, 2].sum()
    return np.asarray(fwd - gold, dtype=f32)[()]


def kernel(feats, transitions, mask, tags):
    feats = np.asarray(feats)
    transitions = np.asarray(transitions)
    mask = np.asarray(mask)
    tags = np.asarray(tags)

    if "nc" not in _CACHE:
        _CACHE["nc"] = _build_nc(debug=False)
    nc = _CACHE["nc"]

    from concourse import bass_utils

    in_maps = [
        _prep_core_inputs(feats, transitions, mask, tags, c) for c in range(NCORES)
    ]
    out = bass_utils.run_bass_kernel_spmd(nc, in_maps, core_ids=list(range(NCORES)))
    return _combine(out.results, mask)



# revision 8
# speedup vs baseline: 2.9743x; 2.9743x over previous
"""Trainium2 Bass kernel for CRF negative log-likelihood (nn_CRF), v2.

Strategy:
  - data-parallel over batch: 8 cores x 16 sequences each.
  - forward algorithm in the exp domain with the absorbing-STOP trick
    (emissions gated by the mask; constant per-step rescale exp(-C0)).
  - the 256-step serial chain is broken into 32 chunks of 8 steps using the
    Perron-Frobenius rank-1 property of positive transfer-matrix products:
    each middle chunk i is represented by M_i ~ (M_i w)(z^T M_i)/(z^T M_i w),
    so only 8-step-long *independent* vector scans are needed.  Validated to
    ~1e-7 rel in fp32 (sim_v2.py); bf16 on device gives ~5e-4.
  - 62 chains (31 fwd + 31 bwd) are packed as 31 "stacks" of a 104-partition
    state [fwd_52; bwd_52] sharing one block-diagonal bf16 weight matrix
    W = blkdiag(exp(T), exp(T)^T), and the stacks are fused column-wise into
    2 super-states (104x224, 104x272): ONE matmul + ONE DVE multiply per
    scan step per super (16 matmuls + 16 muls total for the whole scan).
  - backward chains run in "gamma form" (emission applied after the matmul,
    same as fwd); the glue dot products need one extra matmul with the
    swapped block matrix and a (E 1)-weighted column sum (csA).
  - gold path score via one-hot is_equal tensors built on GPSIMD from
    DMA-replicated masked-tag/prev-tag tensors, contracted with a shared-
    weights PE accumulation loop (32 LDW, 64 MM).
  - device emits raw per-stack dots/weighted-colsums + gold partials; host
    takes logs and sums ("all-reduce").
"""

import numpy as np
import ml_dtypes

BF16 = ml_dtypes.bfloat16

TAG = 52
START, STOP = TAG - 2, TAG - 1
B, S = 128, 256
NCORES = 8
BL = B // NCORES            # 16 sequences per core
L = 8                       # chunk length (scan steps per chain)
NSTACK = 31
C0 = 4.9                    # constant per-step rescale (nats)
MGATE = 64.0                # mask gate constant
NEG = -10000.0
M32 = (S * BL) // 128       # 32 free columns for gold (128, M32) layout

SUP_STACKS = [list(range(1, 15)), list(range(15, 32))]
SUP_NCOL = [224, 272]

_CACHE: dict = {}


def _t_top(i, k):
    return 8 * (i - 1) + 1 + k


def _t_bot(i, k):
    if i == 31:
        return None if k == 0 else 256 - k
    return 8 * i + 8 - k


def _build_nc(debug: bool = False):
    import concourse.bass as bass
    import concourse.mybir as mybir
    import concourse.tile as tile
    from concourse import bacc

    f32 = mybir.dt.float32
    bf16 = mybir.dt.bfloat16
    AL = mybir.AluOpType
    ACT = mybir.ActivationFunctionType

    nc = bacc.Bacc("TRN2", target_bir_lowering=False, debug=debug)

    # ---- external inputs (host-marshalled layouts, per-core shards) ----
    em0d = nc.dram_tensor("em0d", (104, L, 224), bf16, kind="ExternalInput")
    em1d = nc.dram_tensor("em1d", (104, L, 272), bf16, kind="ExternalInput")
    mk1t = nc.dram_tensor("mk1t", (L, 272), bf16, kind="ExternalInput")
    mk1b = nc.dram_tensor("mk1b", (L, 272), bf16, kind="ExternalInput")
    sgated = nc.dram_tensor("sgated", (104, 1), f32, kind="ExternalInput")
    biasAd = nc.dram_tensor("biasAd", (104, 1), f32, kind="ExternalInput")
    biasBd = nc.dram_tensor("biasBd", (104, 1), f32, kind="ExternalInput")
    feats0d = nc.dram_tensor("feats0d", (TAG, BL), f32, kind="ExternalInput")
    transrd = nc.dram_tensor("transrd", (TAG, TAG), f32, kind="ExternalInput")
    identfd = nc.dram_tensor("identfd", (TAG, TAG), f32, kind="ExternalInput")
    tgmfd = nc.dram_tensor("tgmfd", (128, M32), bf16, kind="ExternalInput")
    pvfd = nc.dram_tensor("pvfd", (128, M32), bf16, kind="ExternalInput")
    iotad = nc.dram_tensor("iotad", (128, TAG), bf16, kind="ExternalInput")
    fnwd = nc.dram_tensor("fnwd", (128, M32, TAG + 1), bf16, kind="ExternalInput")
    transTd = nc.dram_tensor("transTd", (TAG, TAG), f32, kind="ExternalInput")
    gcd = nc.dram_tensor("gcd", (TAG, TAG + 1), f32, kind="ExternalInput")
    ginitd = nc.dram_tensor("ginitd", (TAG, BL), bf16, kind="ExternalInput")

    # ---- external outputs ----
    # out_scan cols: [0:224] dots sup0, [224:496] dots sup1,
    #                [496:720] csA-sums sup0, [720:992] csA-sums sup1
    out_scan = nc.dram_tensor("out_scan", (1, 992), f32, kind="ExternalOutput")
    # out_gold: col0 = rowsum(PSE*GC) (emit+end), col1 = rowsum(cntT*transT)
    out_gold = nc.dram_tensor("out_gold", (TAG, 2), f32, kind="ExternalOutput")

    with tile.TileContext(nc) as tc:
        with (
            tc.tile_pool(name="persist", bufs=1) as persist,
            tc.tile_pool(name="chunks", bufs=2) as chunks,
            tc.tile_pool(name="state", bufs=2) as statep,
            tc.tile_pool(name="gold", bufs=1) as goldp,
            tc.tile_pool(name="psum", bufs=1, space="PSUM") as psum,
        ):
            # ================= constants / weights =================
            tr_sb = persist.tile([TAG, TAG], f32, name="tr_sb", tag="tr_sb")
            nc.sync.dma_start(out=tr_sb, in_=transrd[:, :])
            idf = persist.tile([TAG, TAG], f32, name="idf", tag="idf")
            nc.sync.dma_start(out=idf, in_=identfd[:, :])
            colc = persist.tile([104, 3], f32, name="colc", tag="colc")
            nc.sync.dma_start(out=colc[:, 0:1], in_=sgated[:, :])
            nc.sync.dma_start(out=colc[:, 1:2], in_=biasAd[:, :])
            nc.sync.dma_start(out=colc[:, 2:3], in_=biasBd[:, :])
            sgate = colc[:, 0:1]
            biasA = colc[:, 1:2]
            biasB = colc[:, 2:3]
            f0 = persist.tile([TAG, BL], f32, name="f0", tag="f0")
            nc.sync.dma_start(out=f0, in_=feats0d[:, :])

            # Etil = exp(trans) (host pre-patched trans[STOP,STOP]=0)
            etf = persist.tile([TAG, TAG], f32, name="etf", tag="etf")
            nc.scalar.activation(out=etf, in_=tr_sb, func=ACT.Exp)
            etb = persist.tile([TAG, TAG], bf16, name="etb", tag="etb")
            nc.vector.tensor_copy(etb, etf)
            # EtilT via PE transpose (fp32), cast to bf16
            ps_t = psum.tile([TAG, TAG], f32, name="ps_t", tag="ps2")
            nc.tensor.transpose(ps_t, etf, idf)
            etTb = persist.tile([TAG, TAG], bf16, name="etTb", tag="etTb")
            nc.vector.tensor_copy(etTb, ps_t)
            etTf = persist.tile([TAG, TAG], f32, name="etTf", tag="etTf")
            nc.vector.tensor_copy(etTf, ps_t)

            # W = blkdiag(Etil, EtilT) (104, 128) bf16; cols 104.. stay 0
            W = persist.tile([104, 128], bf16, name="W", tag="W")
            nc.vector.memset(W, 0.0)
            nc.vector.tensor_copy(W[0:52, 0:52], etb)
            nc.sync.dma_start(out=W[52:104, 52:104], in_=etTb)  # cross-partition
            # Wsw: rows 52:104 = EtilT, rows 0:52 = 0  (for stage: E @ bwd_state)
            Wsw = persist.tile([104, TAG], bf16, name="Wsw", tag="Wsw")
            nc.vector.memset(Wsw, 0.0)
            nc.sync.dma_start(out=Wsw[52:104, 0:52], in_=etTb)
            # ones / csA = rowsums of Etil (via matmul with lhsT=EtilT)
            ones52 = persist.tile([TAG, 1], bf16, name="ones52", tag="ones52")
            nc.vector.memset(ones52, 1.0)
            ps_r = psum.tile([TAG, 1], f32, name="ps_r", tag="psdot")
            nc.tensor.matmul(ps_r, etTb, ones52, start=True, stop=True)
            csA = persist.tile([TAG, 1], bf16, name="csA", tag="csA")
            nc.vector.tensor_copy(csA, ps_r)

            # ================= state init =================
            st0 = statep.tile([104, 224], bf16, name="st0_init", tag="st0")
            nc.vector.memset(st0, 1.0)
            st1 = statep.tile([104, 272], bf16, name="st1_init", tag="st1")
            nc.vector.memset(st1, 1.0)
            # G (stack 31 = sup1 col block 16) bottom: onehot(STOP) via DMA
            # (engine APs must start at a 32-aligned partition; DMA need not)
            nc.sync.dma_start(out=st1[52:104, 256:272], in_=ginitd[:, :])
            # F (stack 1 top): x0 = exp(f0 - C0) * EtilT[:, START]
            d0 = persist.tile([TAG, BL], f32, name="d0", tag="d0")
            nc.scalar.activation(out=d0, in_=f0, func=ACT.Exp, bias=biasA[0:52, 0:1])
            nc.vector.tensor_scalar_mul(
                out=st0[0:52, 0:BL], in0=d0, scalar1=etTf[:, START : START + 1]
            )

            # ================= emissions =================
            em_sb = [
                persist.tile([104, L, 224], bf16, name="em0", tag="em0"),
                persist.tile([104, L, 272], bf16, name="em1", tag="em1"),
            ]
            for k in range(L):
                # super0: mask-free -> exp(raw + biasA)
                r0 = chunks.tile([104, 224], bf16, name="r0", tag="r0")
                nc.sync.dma_start(out=r0, in_=em0d[:, k, :])
                nc.scalar.activation(
                    out=em_sb[0][:, k, :], in_=r0, func=ACT.Exp, bias=biasA
                )
                # super1: gated -> exp(raw + mask*sgate + biasB)
                r1 = chunks.tile([104, 272], bf16, name="r1", tag="r1")
                nc.sync.dma_start(out=r1, in_=em1d[:, k, :])
                mrep = chunks.tile([104, 272], bf16, name="mrep", tag="mrep")
                nc.sync.dma_start(
                    out=mrep[0:52, :],
                    in_=bass.AP(tensor=mk1t, offset=k * 272, ap=[[0, 52], [1, 272]]),
                )
                nc.sync.dma_start(
                    out=mrep[52:104, :],
                    in_=bass.AP(tensor=mk1b, offset=k * 272, ap=[[0, 52], [1, 272]]),
                )
                nc.vector.scalar_tensor_tensor(
                    out=r1, in0=mrep, scalar=sgate, in1=r1, op0=AL.mult, op1=AL.add
                )
                nc.scalar.activation(
                    out=em_sb[1][:, k, :], in_=r1, func=ACT.Exp, bias=biasB
                )

            # ================= fused scan =================
            st = [st0, st1]
            for k in range(L):
                for sup in (0, 1):
                    ncol = SUP_NCOL[sup]
                    ps = psum.tile(
                        [128, ncol], f32, name=f"ps{sup}_{k}", tag=f"pss{sup}"
                    )
                    nc.tensor.matmul(ps, W, st[sup], start=True, stop=True)
                    stn = statep.tile(
                        [104, ncol], bf16, name=f"st{sup}_{k}", tag=f"st{sup}"
                    )
                    nc.vector.tensor_tensor(
                        out=stn, in0=ps[0:104, :], in1=em_sb[sup][:, k, :], op=AL.mult
                    )
                    st[sup] = stn

            # ================= stage (dots + weighted colsums) =================
            stage = persist.tile([1, 992], f32, name="stage", tag="stage")
            for sup in (0, 1):
                ncol = SUP_NCOL[sup]
                off = 0 if sup == 0 else 224
                ps2 = psum.tile([TAG, ncol], f32, name=f"ps2_{sup}", tag="ps2")
                nc.tensor.matmul(ps2, Wsw, st[sup], start=True, stop=True)
                dotp = chunks.tile([TAG, ncol], bf16, name=f"dotp{sup}", tag="dotp")
                nc.vector.tensor_tensor(
                    out=dotp, in0=ps2, in1=st[sup][0:52, :], op=AL.mult
                )
                ps_d = psum.tile([1, ncol], f32, name=f"psd{sup}", tag="psdot")
                nc.tensor.matmul(ps_d, ones52, dotp, start=True, stop=True)
                nc.scalar.copy(stage[0:1, off : off + ncol], ps_d)
                ps_c = psum.tile([1, ncol], f32, name=f"psc{sup}", tag="psden")
                nc.tensor.matmul(ps_c, csA, st[sup][0:52, :], start=True, stop=True)
                nc.scalar.copy(stage[0:1, 496 + off : 496 + off + ncol], ps_c)
            nc.sync.dma_start(out=out_scan[:, :], in_=stage)

            # ================= gold =================
            io_sb = goldp.tile([128, TAG], bf16, name="io_sb", tag="io_sb")
            nc.sync.dma_start(out=io_sb, in_=iotad[:, :])
            tgm_sb = goldp.tile([128, M32], bf16, name="tgm_sb", tag="tgm_sb")
            nc.sync.dma_start(out=tgm_sb, in_=tgmfd[:, :])
            pv_sb = goldp.tile([128, M32], bf16, name="pv_sb", tag="pv_sb")
            nc.sync.dma_start(out=pv_sb, in_=pvfd[:, :])
            fnw_sb = goldp.tile([128, M32, TAG + 1], bf16, name="fnw_sb", tag="fnw_sb")
            nc.sync.dma_start(out=fnw_sb, in_=fnwd[:, :, :])
            trT = goldp.tile([TAG, TAG], f32, name="trT", tag="trT")
            nc.sync.dma_start(out=trT, in_=transTd[:, :])
            gc_sb = goldp.tile([TAG, TAG + 1], f32, name="gc_sb", tag="gc_sb")
            nc.sync.dma_start(out=gc_sb, in_=gcd[:, :])

            io3 = bass.AP(
                tensor=io_sb.tensor,
                offset=io_sb.offset,
                ap=[io_sb.ap[0], [0, M32], io_sb.ap[1]],
            )

            def b3(v):
                return bass.AP(
                    tensor=v.tensor, offset=v.offset, ap=[v.ap[0], v.ap[1], [0, TAG]]
                )

            Y = goldp.tile([128, M32, TAG], bf16, name="Y", tag="Y")
            nc.vector.tensor_tensor(out=Y, in0=b3(tgm_sb), in1=io3, op=AL.is_equal)
            YP = goldp.tile([128, M32, TAG], bf16, name="YP", tag="YP")
            nc.vector.tensor_tensor(out=YP, in0=b3(pv_sb), in1=io3, op=AL.is_equal)

            ps_cnt = psum.tile([TAG, TAG], f32, name="ps_cnt", tag="ps_cnt")
            ps_e = psum.tile([TAG, TAG + 1], f32, name="ps_e", tag="ps_e")
            for m in range(M32):
                nc.tensor.matmul(
                    ps_cnt,
                    Y[:, m, :],
                    YP[:, m, :],
                    start=(m == 0),
                    stop=(m == M32 - 1),
                )
                nc.tensor.matmul(
                    ps_e,
                    Y[:, m, :],
                    fnw_sb[:, m, :],
                    start=(m == 0),
                    stop=(m == M32 - 1),
                )

            stage_g = goldp.tile([TAG, 2], f32, name="stage_g", tag="stage_g")
            scr1 = goldp.tile([TAG, TAG], f32, name="scr1", tag="scr1")
            nc.vector.tensor_tensor(out=scr1, in0=ps_cnt, in1=trT, op=AL.mult)
            nc.vector.tensor_reduce(
                out=stage_g[:, 1:2], in_=scr1, axis=mybir.AxisListType.X, op=AL.add
            )
            scr2 = goldp.tile([TAG, TAG + 1], f32, name="scr2", tag="scr2")
            nc.vector.tensor_tensor(out=scr2, in0=ps_e, in1=gc_sb, op=AL.mult)
            nc.vector.tensor_reduce(
                out=stage_g[:, 0:1], in_=scr2, axis=mybir.AxisListType.X, op=AL.add
            )
            nc.sync.dma_start(out=out_gold[:, :], in_=stage_g)

    nc.compile()
    return nc


def _prep_core_inputs(feats, transitions, mask, tags, core):
    """Host-side layout marshalling of the core's batch shard."""
    f32 = np.float32
    sl = slice(core * BL, (core + 1) * BL)
    f = np.ascontiguousarray(feats[sl]).astype(f32)        # (BL,S,T)
    m = mask[sl].astype(f32)                               # (BL,S)
    tg = tags[sl].astype(np.int64)                         # (BL,S)

    out = {}
    for sup in (0, 1):
        stacks = SUP_STACKS[sup]
        ncol = SUP_NCOL[sup]
        em = np.zeros((104, L, ncol), f32)
        mk_top = np.ones((L, ncol), f32)
        mk_bot = np.ones((L, ncol), f32)
        for ci, i in enumerate(stacks):
            cs = slice(ci * BL, (ci + 1) * BL)
            for k in range(L):
                tt = _t_top(i, k)
                em[0:52, k, cs] = f[:, tt, :].T
                em[STOP, k, cs] = NEG if sup == 0 else 0.0
                mk_top[k, cs] = m[:, tt]
                tb = _t_bot(i, k)
                if tb is None:                             # G pad: d = onehot(STOP)
                    em[52:103, k, cs] = NEG
                    em[103, k, cs] = 0.0
                    mk_bot[k, cs] = 0.0
                else:
                    em[52:104, k, cs] = f[:, tb, :].T
                    em[52 + STOP, k, cs] = NEG if sup == 0 else 0.0
                    mk_bot[k, cs] = m[:, tb]
        out[f"em{sup}d"] = np.ascontiguousarray(em).astype(BF16)
        if sup == 1:
            out["mk1t"] = np.ascontiguousarray(mk_top).astype(BF16)
            out["mk1b"] = np.ascontiguousarray(mk_bot).astype(BF16)

    sgate = np.full((104, 1), MGATE, f32)
    sgate[STOP, 0] = -MGATE
    sgate[52 + STOP, 0] = -MGATE
    out["sgated"] = sgate
    out["biasAd"] = np.full((104, 1), -C0, f32)
    biasB = np.full((104, 1), -(MGATE + C0), f32)
    biasB[STOP, 0] = 0.0
    biasB[52 + STOP, 0] = 0.0
    out["biasBd"] = biasB
    feats0 = np.ascontiguousarray(f[:, 0, :].T).copy()
    feats0[STOP, :] = NEG
    out["feats0d"] = feats0
    trp = transitions.astype(f32).copy()
    trp[STOP, STOP] = 0.0                                  # E[STOP,STOP] = 1
    out["transrd"] = trp
    out["identfd"] = np.eye(TAG, dtype=f32)

    tgm = (tg + 1) * mask[sl].astype(np.int64) - 1
    out["tgmfd"] = tgm.reshape(128, M32).astype(f32).astype(BF16)
    prev = np.concatenate([np.full((BL, 1), START, np.int64), tg[:, :-1]], axis=1)
    out["pvfd"] = prev.reshape(128, M32).astype(f32).astype(BF16)
    mnext = np.concatenate([m[:, 1:], np.zeros((BL, 1), f32)], axis=1)
    wl = (m - mnext).reshape(128, M32)
    fnw = np.zeros((128, M32, TAG + 1), f32)
    fnw[:, :, 0:TAG] = f.reshape(128, M32, TAG)
    fnw[:, :, TAG] = wl
    out["fnwd"] = fnw.astype(BF16)
    out["iotad"] = (
        np.broadcast_to(np.arange(TAG, dtype=f32), (128, TAG)).copy().astype(BF16)
    )
    out["transTd"] = transitions.T.astype(f32).copy()
    gc = np.zeros((TAG, TAG + 1), f32)
    gc[:, 0:TAG] = np.eye(TAG, dtype=f32)
    gc[:, TAG] = transitions[:, STOP]
    out["gcd"] = gc
    ginit = np.zeros((TAG, BL), f32)
    ginit[STOP, :] = 1.0
    out["ginitd"] = ginit.astype(BF16)
    return out


def _combine(results, mask):
    """Host-side unshard: logs of staged dots/denominators -> scalar."""
    lengths = mask.astype(np.int64).sum(axis=1)
    fwd = np.float64(0.0)
    gold = np.float64(0.0)
    for core, res in enumerate(results):
        sc = res["out_scan"].astype(np.float64).reshape(992)
        gl = res["out_gold"].astype(np.float64)            # (52, 2)
        dots = np.concatenate([sc[0:224], sc[224:496]]).reshape(NSTACK, BL)
        dens = np.concatenate([sc[496:720], sc[720:992]]).reshape(NSTACK, BL)
        ln = np.log(dots).sum(axis=0) - np.log(dens[1:NSTACK]).sum(axis=0)
        lens = lengths[core * BL : (core + 1) * BL].astype(np.float64)
        fwd += (ln + C0 * lens).sum()
        gold += gl.sum()
    return np.float32(fwd - gold)


def kernel(feats, transitions, mask, tags):
    feats = np.asarray(feats)
    transitions = np.asarray(transitions)
    mask = np.asarray(mask)
    tags = np.asarray(tags)

    if "nc" not in _CACHE:
        _CACHE["nc"] = _build_nc(debug=False)
    nc = _CACHE["nc"]

    from concourse import bass_utils

    in_maps = [
        _prep_core_inputs(feats, transitions, mask, tags, c) for c in range(NCORES)
    ]
    out = bass_utils.run_bass_kernel_spmd(nc, in_maps, core_ids=list(range(NCORES)))
    return _combine(out.results, mask)


# revision 9
# speedup vs baseline: 4.5439x; 1.5277x over previous
"""Trainium2 Bass kernel for CRF negative log-likelihood (nn_CRF), v3.

Strategy:
  - data-parallel over batch: 8 cores x 16 sequences each.
  - forward algorithm in the exp domain with the absorbing-STOP trick
    (emissions gated by the mask; constant per-step rescale exp(-C0)).
  - the 256-step serial chain is broken into 32 chunks of 8 steps using the
    Perron-Frobenius rank-1 property of positive transfer-matrix products:
    each middle chunk i is represented by M_i ~ (M_i w)(z^T M_i)/(z^T M_i w),
    so only 8-step-long *independent* vector scans are needed.  Validated to
    ~1e-7 rel in fp32 (sim_v2.py); bf16 on device gives ~5e-4.
  - 62 chains (31 fwd + 31 bwd) are packed as 31 "stacks" of a 104-partition
    state [fwd_52; bwd_52] sharing one block-diagonal bf16 weight matrix
    W = blkdiag(exp(T), exp(T)^T), and the stacks are fused column-wise into
    2 super-states (104x224, 104x272): ONE matmul + ONE DVE multiply per
    scan step per super (16 matmuls + 16 muls total for the whole scan).
  - backward chains run in "gamma form" (emission applied after the matmul,
    same as fwd); the glue dot products need one extra matmul with the
    swapped block (= W[:, 52:104]) and a (E 1)-weighted column sum (csA).
  - gold path score via one-hot is_equal tensors contracted with a shared-
    weights PE accumulation loop; gold runs BEFORE the scan (fills the
    startup window, warms the PE clock gate).
  - DMAs are consolidated (one per logical tensor, inputs packed) and issued
    from both the Sync and GpSimd queues: each dma_start costs ~770ns of
    issuing-engine time, which was the v2 bottleneck.
  - device emits raw per-stack dots/weighted-colsums + gold partials; host
    takes logs and sums ("all-reduce").
"""

import numpy as np
import ml_dtypes

BF16 = ml_dtypes.bfloat16

TAG = 52
START, STOP = TAG - 2, TAG - 1
B, S = 128, 256
NCORES = 8
BL = B // NCORES            # 16 sequences per core
L = 8                       # chunk length (scan steps per chain)
NSTACK = 31
C0 = 4.9                    # constant per-step rescale (nats)
MGATE = 64.0                # mask gate constant
NEG = -10000.0
M32 = (S * BL) // 128       # 32 free columns for gold (128, M32) layout

SUP_STACKS = [list(range(1, 15)), list(range(15, 32))]
SUP_NCOL = [224, 272]

_CACHE: dict = {}


def _t_top(i, k):
    return 8 * (i - 1) + 1 + k


def _t_bot(i, k):
    if i == 31:
        return None if k == 0 else 256 - k
    return 8 * i + 8 - k


def _build_nc(debug: bool = False):
    import concourse.bass as bass
    import concourse.mybir as mybir
    import concourse.tile as tile
    from concourse import bacc

    f32 = mybir.dt.float32
    bf16 = mybir.dt.bfloat16
    AL = mybir.AluOpType
    ACT = mybir.ActivationFunctionType

    nc = bacc.Bacc("TRN2", target_bir_lowering=False, debug=debug)

    # ---- external inputs (host-marshalled layouts, per-core shards) ----
    em0d = nc.dram_tensor("em0d", (104, L, 224), bf16, kind="ExternalInput")
    em1d = nc.dram_tensor("em1d", (104, L, 272), bf16, kind="ExternalInput")
    mk1t = nc.dram_tensor("mk1t", (L, 272), bf16, kind="ExternalInput")
    mk1b = nc.dram_tensor("mk1b", (L, 272), bf16, kind="ExternalInput")
    # colcd cols: 0=sgate, 1=biasA, 2=biasB
    colcd = nc.dram_tensor("colcd", (104, 3), f32, kind="ExternalInput")
    # setupA cols: [0:52]=trans (STOP,STOP patched 0), [52:104]=ident,
    #              [104:120]=feats0 (STOP row NEG)
    setupAd = nc.dram_tensor("setupAd", (TAG, 120), f32, kind="ExternalInput")
    ginitd = nc.dram_tensor("ginitd", (TAG, BL), bf16, kind="ExternalInput")
    # gpack cols: [0:32]=tgm, [32:64]=prev, [64:116]=iota
    gpackd = nc.dram_tensor("gpackd", (128, 116), bf16, kind="ExternalInput")
    fnwd = nc.dram_tensor("fnwd", (128, M32, TAG + 1), bf16, kind="ExternalInput")
    # gf32 cols: [0:52]=trans.T, [52:105]=GC (ident | trans[:,STOP])
    gf32d = nc.dram_tensor("gf32d", (TAG, 105), f32, kind="ExternalInput")

    # ---- external outputs ----
    out_scan = nc.dram_tensor("out_scan", (1, 992), f32, kind="ExternalOutput")
    out_gold = nc.dram_tensor("out_gold", (TAG, 2), f32, kind="ExternalOutput")

    with tile.TileContext(nc) as tc:
        with (
            tc.tile_pool(name="persist", bufs=1) as persist,
            tc.tile_pool(name="chunks", bufs=2) as chunks,
            tc.tile_pool(name="state", bufs=2) as statep,
            tc.tile_pool(name="gold", bufs=1) as goldp,
            tc.tile_pool(name="psum", bufs=1, space="PSUM") as psum,
        ):
            # ================= constants / weights =================
            setupA = persist.tile([TAG, 120], f32, name="setupA", tag="setupA")
            nc.sync.dma_start(out=setupA, in_=setupAd[:, :])
            tr_sb = setupA[:, 0:52]
            idf = setupA[:, 52:104]
            f0 = setupA[:, 104:120]
            colc = persist.tile([104, 3], f32, name="colc", tag="colc")
            nc.sync.dma_start(out=colc, in_=colcd[:, :])
            sgate = colc[:, 0:1]
            biasA = colc[:, 1:2]
            biasB = colc[:, 2:3]

            etf = persist.tile([TAG, TAG], f32, name="etf", tag="etf")
            nc.scalar.activation(out=etf, in_=tr_sb, func=ACT.Exp)
            etb = persist.tile([TAG, TAG], bf16, name="etb", tag="etb")
            nc.vector.tensor_copy(etb, etf)
            ps_t = psum.tile([TAG, TAG], f32, name="ps_t", tag="ps2")
            nc.tensor.transpose(ps_t, etf, idf)
            etTb = persist.tile([TAG, TAG], bf16, name="etTb", tag="etTb")
            nc.vector.tensor_copy(etTb, ps_t)
            etTf = persist.tile([TAG, TAG], f32, name="etTf", tag="etTf")
            nc.vector.tensor_copy(etTf, ps_t)

            # W = blkdiag(Etil, EtilT) (104, 128) bf16.
            # W[:, 52:104] doubles as the stage weight (E @ bwd_state).
            W = persist.tile([104, 128], bf16, name="W", tag="W")
            nc.vector.memset(W, 0.0)
            nc.vector.tensor_copy(W[0:52, 0:52], etb)
            nc.sync.dma_start(out=W[52:104, 52:104], in_=etTb)  # cross-partition
            ones52 = persist.tile([TAG, 1], bf16, name="ones52", tag="ones52")
            nc.vector.memset(ones52, 1.0)
            ps_r = psum.tile([TAG, 1], f32, name="ps_r", tag="psdot")
            nc.tensor.matmul(ps_r, etTb, ones52, start=True, stop=True)
            csA = persist.tile([TAG, 1], bf16, name="csA", tag="csA")
            nc.vector.tensor_copy(csA, ps_r)

            # ================= state init =================
            st0 = statep.tile([104, 224], bf16, name="st0_init", tag="st0")
            nc.vector.memset(st0, 1.0)
            st1 = statep.tile([104, 272], bf16, name="st1_init", tag="st1")
            nc.vector.memset(st1, 1.0)
            # G (stack 31) bottom: onehot(STOP) via DMA (engine APs must be
            # 32-aligned in partition start; DMA need not)
            nc.sync.dma_start(out=st1[52:104, 256:272], in_=ginitd[:, :])
            d0 = persist.tile([TAG, BL], f32, name="d0", tag="d0")
            nc.scalar.activation(out=d0, in_=f0, func=ACT.Exp, bias=biasA[0:52, 0:1])
            nc.vector.tensor_scalar_mul(
                out=st0[0:52, 0:BL], in0=d0, scalar1=etTf[:, START : START + 1]
            )

            # ================= gold (runs before/under the scan) =============
            gpack = goldp.tile([128, 116], bf16, name="gpack", tag="gpack")
            nc.gpsimd.dma_start(out=gpack, in_=gpackd[:, :])
            fnw_sb = goldp.tile([128, M32, TAG + 1], bf16, name="fnw_sb", tag="fnw_sb")
            nc.gpsimd.dma_start(out=fnw_sb, in_=fnwd[:, :, :])
            gf32 = goldp.tile([TAG, 105], f32, name="gf32", tag="gf32")
            nc.gpsimd.dma_start(out=gf32, in_=gf32d[:, :])
            trT = gf32[:, 0:52]
            gc_sb = gf32[:, 52:105]

            io3 = bass.AP(
                tensor=gpack.tensor,
                offset=gpack.offset + 64,
                ap=[gpack.ap[0], [0, M32], [1, TAG]],
            )

            def b3(off):
                return bass.AP(
                    tensor=gpack.tensor,
                    offset=gpack.offset + off,
                    ap=[gpack.ap[0], [1, M32], [0, TAG]],
                )

            Y = goldp.tile([128, M32, TAG], bf16, name="Y", tag="Y")
            nc.vector.tensor_tensor(out=Y, in0=b3(0), in1=io3, op=AL.is_equal)
            YP = goldp.tile([128, M32, TAG], bf16, name="YP", tag="YP")
            nc.vector.tensor_tensor(out=YP, in0=b3(32), in1=io3, op=AL.is_equal)

            ps_cnt = psum.tile([TAG, TAG], f32, name="ps_cnt", tag="ps_cnt")
            ps_e = psum.tile([TAG, TAG + 1], f32, name="ps_e", tag="ps_e")
            for m in range(M32):
                nc.tensor.matmul(
                    ps_cnt,
                    Y[:, m, :],
                    YP[:, m, :],
                    start=(m == 0),
                    stop=(m == M32 - 1),
                )
                nc.tensor.matmul(
                    ps_e,
                    Y[:, m, :],
                    fnw_sb[:, m, :],
                    start=(m == 0),
                    stop=(m == M32 - 1),
                )
            stage_g = goldp.tile([TAG, 2], f32, name="stage_g", tag="stage_g")
            scr1 = goldp.tile([TAG, TAG], f32, name="scr1", tag="scr1")
            nc.vector.tensor_tensor(out=scr1, in0=ps_cnt, in1=trT, op=AL.mult)
            nc.vector.tensor_reduce(
                out=stage_g[:, 1:2], in_=scr1, axis=mybir.AxisListType.X, op=AL.add
            )
            scr2 = goldp.tile([TAG, TAG + 1], f32, name="scr2", tag="scr2")
            nc.vector.tensor_tensor(out=scr2, in0=ps_e, in1=gc_sb, op=AL.mult)
            nc.vector.tensor_reduce(
                out=stage_g[:, 0:1], in_=scr2, axis=mybir.AxisListType.X, op=AL.add
            )
            nc.gpsimd.dma_start(out=out_gold[:, :], in_=stage_g)

            # ================= emissions =================
            em_sb = [
                persist.tile([104, L, 224], bf16, name="em0", tag="em0"),
                persist.tile([104, L, 272], bf16, name="em1", tag="em1"),
            ]
            nc.sync.dma_start(out=em_sb[0], in_=em0d[:, :, :])
            raw1 = persist.tile([104, L, 272], bf16, name="raw1", tag="raw1")
            nc.sync.dma_start(out=raw1, in_=em1d[:, :, :])
            mrep = persist.tile([104, L, 272], bf16, name="mrep", tag="mrep")
            nc.sync.dma_start(
                out=mrep[0:52, :, :],
                in_=bass.AP(tensor=mk1t, offset=0, ap=[[0, 52], [1, L * 272]]),
            )
            nc.sync.dma_start(
                out=mrep[52:104, :, :],
                in_=bass.AP(tensor=mk1b, offset=0, ap=[[0, 52], [1, L * 272]]),
            )
            # build in 2 halves (k 0:4, 4:8) so the scan can start early
            for h in (0, 1):
                ks = slice(h * 4, h * 4 + 4)
                nc.scalar.activation(
                    out=em_sb[0][:, ks, :], in_=em_sb[0][:, ks, :],
                    func=ACT.Exp, bias=biasA,
                )
                nc.vector.scalar_tensor_tensor(
                    out=raw1[:, ks, :], in0=mrep[:, ks, :], scalar=sgate,
                    in1=raw1[:, ks, :], op0=AL.mult, op1=AL.add,
                )
                nc.scalar.activation(
                    out=em_sb[1][:, ks, :], in_=raw1[:, ks, :],
                    func=ACT.Exp, bias=biasB,
                )

            # ================= fused scan =================
            st = [st0, st1]
            for k in range(L):
                for sup in (0, 1):
                    ncol = SUP_NCOL[sup]
                    ps = psum.tile(
                        [128, ncol], f32, name=f"ps{sup}_{k}", tag=f"pss{sup}"
                    )
                    nc.tensor.matmul(ps, W, st[sup], start=True, stop=True)
                    stn = statep.tile(
                        [104, ncol], bf16, name=f"st{sup}_{k}", tag=f"st{sup}"
                    )
                    nc.vector.tensor_tensor(
                        out=stn, in0=ps[0:104, :], in1=em_sb[sup][:, k, :], op=AL.mult
                    )
                    st[sup] = stn

            # ================= stage (dots + weighted colsums) ===============
            stage = persist.tile([1, 992], f32, name="stage", tag="stage")
            for sup in (0, 1):
                ncol = SUP_NCOL[sup]
                off = 0 if sup == 0 else 224
                ps2 = psum.tile([TAG, ncol], f32, name=f"ps2_{sup}", tag="ps2")
                nc.tensor.matmul(ps2, W[:, 52:104], st[sup], start=True, stop=True)
                dotp = chunks.tile([TAG, ncol], bf16, name=f"dotp{sup}", tag="dotp")
                nc.vector.tensor_tensor(
                    out=dotp, in0=ps2, in1=st[sup][0:52, :], op=AL.mult
                )
                ps_d = psum.tile([1, ncol], f32, name=f"psd{sup}", tag="psdot")
                nc.tensor.matmul(ps_d, ones52, dotp, start=True, stop=True)
                nc.scalar.copy(stage[0:1, off : off + ncol], ps_d)
                ps_c = psum.tile([1, ncol], f32, name=f"psc{sup}", tag="psden")
                nc.tensor.matmul(ps_c, csA, st[sup][0:52, :], start=True, stop=True)
                nc.scalar.copy(stage[0:1, 496 + off : 496 + off + ncol], ps_c)
            nc.sync.dma_start(out=out_scan[:, :], in_=stage)

    nc.compile()
    return nc


def _prep_core_inputs(feats, transitions, mask, tags, core):
    """Host-side layout marshalling of the core's batch shard."""
    f32 = np.float32
    sl = slice(core * BL, (core + 1) * BL)
    f = np.ascontiguousarray(feats[sl]).astype(f32)        # (BL,S,T)
    m = mask[sl].astype(f32)                               # (BL,S)
    tg = tags[sl].astype(np.int64)                         # (BL,S)

    out = {}
    for sup in (0, 1):
        stacks = SUP_STACKS[sup]
        ncol = SUP_NCOL[sup]
        em = np.zeros((104, L, ncol), f32)
        mk_top = np.ones((L, ncol), f32)
        mk_bot = np.ones((L, ncol), f32)
        for ci, i in enumerate(stacks):
            cs = slice(ci * BL, (ci + 1) * BL)
            for k in range(L):
                tt = _t_top(i, k)
                em[0:52, k, cs] = f[:, tt, :].T
                em[STOP, k, cs] = NEG if sup == 0 else 0.0
                mk_top[k, cs] = m[:, tt]
                tb = _t_bot(i, k)
                if tb is None:                             # G pad: d = onehot(STOP)
                    em[52:103, k, cs] = NEG
                    em[103, k, cs] = 0.0
                    mk_bot[k, cs] = 0.0
                else:
                    em[52:104, k, cs] = f[:, tb, :].T
                    em[52 + STOP, k, cs] = NEG if sup == 0 else 0.0
                    mk_bot[k, cs] = m[:, tb]
        out[f"em{sup}d"] = np.ascontiguousarray(em).astype(BF16)
        if sup == 1:
            out["mk1t"] = np.ascontiguousarray(mk_top).astype(BF16)
            out["mk1b"] = np.ascontiguousarray(mk_bot).astype(BF16)

    colc = np.zeros((104, 3), f32)
    colc[:, 0] = MGATE
    colc[STOP, 0] = -MGATE
    colc[52 + STOP, 0] = -MGATE
    colc[:, 1] = -C0
    colc[:, 2] = -(MGATE + C0)
    colc[STOP, 2] = 0.0
    colc[52 + STOP, 2] = 0.0
    out["colcd"] = colc

    setupA = np.zeros((TAG, 120), f32)
    trp = transitions.astype(f32).copy()
    trp[STOP, STOP] = 0.0                                  # E[STOP,STOP] = 1
    setupA[:, 0:52] = trp
    setupA[:, 52:104] = np.eye(TAG, dtype=f32)
    feats0 = np.ascontiguousarray(f[:, 0, :].T).copy()
    feats0[STOP, :] = NEG
    setupA[:, 104:120] = feats0
    out["setupAd"] = setupA

    ginit = np.zeros((TAG, BL), f32)
    ginit[STOP, :] = 1.0
    out["ginitd"] = ginit.astype(BF16)

    gpack = np.zeros((128, 116), f32)
    tgm = (tg + 1) * mask[sl].astype(np.int64) - 1
    gpack[:, 0:M32] = tgm.reshape(128, M32)
    prev = np.concatenate([np.full((BL, 1), START, np.int64), tg[:, :-1]], axis=1)
    gpack[:, M32 : 2 * M32] = prev.reshape(128, M32)
    gpack[:, 2 * M32 : 2 * M32 + TAG] = np.arange(TAG, dtype=f32)[None, :]
    out["gpackd"] = gpack.astype(BF16)

    mnext = np.concatenate([m[:, 1:], np.zeros((BL, 1), f32)], axis=1)
    wl = (m - mnext).reshape(128, M32)
    fnw = np.zeros((128, M32, TAG + 1), f32)
    fnw[:, :, 0:TAG] = f.reshape(128, M32, TAG)
    fnw[:, :, TAG] = wl
    out["fnwd"] = fnw.astype(BF16)

    gf32 = np.zeros((TAG, 105), f32)
    gf32[:, 0:52] = transitions.T
    gf32[:, 52:104] = np.eye(TAG, dtype=f32)
    gf32[:, 104] = transitions[:, STOP]
    out["gf32d"] = gf32
    return out


def _combine(results, mask):
    """Host-side unshard: logs of staged dots/denominators -> scalar."""
    lengths = mask.astype(np.int64).sum(axis=1)
    fwd = np.float64(0.0)
    gold = np.float64(0.0)
    for core, res in enumerate(results):
        sc = res["out_scan"].astype(np.float64).reshape(992)
        gl = res["out_gold"].astype(np.float64)            # (52, 2)
        dots = np.concatenate([sc[0:224], sc[224:496]]).reshape(NSTACK, BL)
        dens = np.concatenate([sc[496:720], sc[720:992]]).reshape(NSTACK, BL)
        ln = np.log(dots).sum(axis=0) - np.log(dens[1:NSTACK]).sum(axis=0)
        lens = lengths[core * BL : (core + 1) * BL].astype(np.float64)
        fwd += (ln + C0 * lens).sum()
        gold += gl.sum()
    return np.float32(fwd - gold)


def kernel(feats, transitions, mask, tags):
    feats = np.asarray(feats)
    transitions = np.asarray(transitions)
    mask = np.asarray(mask)
    tags = np.asarray(tags)

    if "nc" not in _CACHE:
        _CACHE["nc"] = _build_nc(debug=False)
    nc = _CACHE["nc"]

    from concourse import bass_utils

    in_maps = [
        _prep_core_inputs(feats, transitions, mask, tags, c) for c in range(NCORES)
    ]
    out = bass_utils.run_bass_kernel_spmd(nc, in_maps, core_ids=list(range(NCORES)))
    return _combine(out.results, mask)
